# revision 59
# baseline (speedup 1.0000x reference)
"""EnergyPool2d Trainium2 kernel, v7.

For each 3x3 sliding window (stride 1, no padding) of each (n,c) image
plane, scatter-add +1 at the window's argmax position and -1 at the
argmin position (first-occurrence, row-major within the window).

Design (v4..v7 evolution from the 310us v2 baseline):
 * planes-on-partitions: 128 (n,c) planes per core, one per SBUF
   partition; all row/col shifts are free-dim AP offsets.
 * fp16 compares (DVE 2x path); fp16 rounding keeps rel_err ~1.5e-2
   < 2e-2 gate, bit-identical winners to the v2 mask algebra.
 * DVE computes only compares/maxes + the vertical T field:
     S = 3-max of row triples, C/D vertical ge-masks, T via
     C*(D+P) + P*Q with P,Q complements on Act (GpSimd tensor ops
     measurably slow concurrent DVE - Pool only does memsets/DMA).
   Per-path P/Q + single-width t12 so the max path's T never waits
   on the min path's compares.
 * The horizontal combine runs on PE + Act via relu-of-linear-sums
   (verified bit-exact in numpy):
     U1[v] = H1[v]*T[v] = relu(T - 3c[v] + 3c[v+1] - 3)
     U2[v] = H2[v]*T[v] = relu(T - 3d[v] - 3c[v+1])
     count[j] = T[j] - U1[j] + U1[j-1] - U2[j] + U2[j-2]
   (min path mirrors with raw is_gt masks g,h and biases -3/-6),
   using sum_b H_b = 1 so H0*T is never materialized.  PE accumulates
   the z-sums with +-I/+-3I stationaries into PSUM, Act relus into
   SBUF U-tiles (the two bias=-3 fields share one wide relu), PE
   accumulates the 10-term count in PSUM, Act copies out.
 * input f32->fp16 conversion happens in-flight in the DMA (GpSimd
   SWDGE cast) - no staging buffer, no Act convert.
 * combine runs in 4-row chunks: PSUM = z1ab (2 banks) + z2a + z2b
   + cnt (bufs=2) = 6 banks.

Data-parallel: 1024 (n,c) planes, 128 per core, 8 cores, no cross-core
communication.
"""

import numpy as np

import concourse.bacc as bacc
import concourse.tile as tile
import concourse.mybir as mybir
from concourse import bass_utils

N_, C_, H, W = 16, 64, 128, 128
NCORES = 8
P = N_ * C_ // NCORES        # 128 planes per core = partition dim
RB = 32                      # rows per block
NBLK = H // RB
CH = 4                       # combine chunk rows
NCH = RB // CH

F32 = mybir.dt.float32
F16 = mybir.dt.float16
BF16 = mybir.dt.bfloat16
Alu = mybir.AluOpType
Act = mybir.ActivationFunctionType


def _cmp_phase(nc, t, blk, is_max):
    """S (3-max/min of rows) and vertical masks C, D for one path, then
    queue the Act complements P, Q (both halves) for the T build."""
    v = nc.vector
    a = nc.scalar
    top, bot = blk == 0, blk == NBLK - 1
    op3 = Alu.max if is_max else Alu.min
    ge = Alu.is_ge if is_max else Alu.is_le
    xh, S = t["xh"], t["S"]
    h = 0 if is_max else 1
    C = t["C12"][:, h]
    D = t["D12"][:, h]

    s0 = 2 if top else 0
    nr = 34 if (top or bot) else 36
    if blk == 0 and is_max:
        # cold start: sub-ops aligned to the load chunks so the first
        # compare starts as early as possible
        for a0, a1 in ((2, 8), (8, 18), (18, 28), (28, 36)):
            v.tensor_tensor(S[:, a0:a1], xh[:, a0:a1, 0:126],
                            xh[:, a0:a1, 1:127], op3)
            v.tensor_tensor(S[:, a0:a1], S[:, a0:a1], xh[:, a0:a1, 2:128], op3)
    else:
        sl = slice(s0, s0 + nr)
        v.tensor_tensor(S[:, sl], xh[:, sl, 0:126], xh[:, sl, 1:127], op3)
        v.tensor_tensor(S[:, sl], S[:, sl], xh[:, sl, 2:128], op3)

    # C[k] ~ C[r0-1+k] (33 rows), D[k] ~ D[r0-2+k] (34 rows)
    if top:
        nc.gpsimd.memset(C[:, 0:1], 1.0)     # C[-1] = 1
        nc.gpsimd.memset(D[:, 0:2], 1.0)     # D[-2] = D[-1] = 1
        v.tensor_tensor(C[:, 1:33], S[:, 2:34], S[:, 3:35], ge)
        v.tensor_tensor(D[:, 2:34], S[:, 2:34], S[:, 4:36], ge)
    elif bot:
        v.tensor_tensor(C[:, 0:32], S[:, 1:33], S[:, 2:34], ge)
        v.tensor_tensor(D[:, 0:32], S[:, 0:32], S[:, 2:34], ge)
        nc.gpsimd.memset(C[:, 32:33], 0.0)   # C[127] = 0
        nc.gpsimd.memset(D[:, 32:34], 0.0)   # D[126] = D[127] = 0
    else:
        v.tensor_tensor(C[:, 0:33], S[:, 1:34], S[:, 2:35], ge)
        v.tensor_tensor(D[:, 0:34], S[:, 0:34], S[:, 2:36], ge)

    # P[i] = 1 - C[i-1], Q[i] = 1 - D[i-2], per 16-row half
    for half in range(2):
        h0 = 16 * half
        a.activation(t["Pq"][half][:, h, 0], C[:, h0:h0 + 16], Act.Identity,
                     bias=1.0, scale=-1.0)
        a.activation(t["Pq"][half][:, h, 1], D[:, h0:h0 + 16], Act.Identity,
                     bias=1.0, scale=-1.0)


def _h_phase(nc, t, r0, r1):
    """Raw horizontal winner masks c,d (is_ge, max) / m,n (is_le, min)
    on rows [r0, r1).  With le-masks the min path's z-forms and relu
    biases are identical to the max path's (m = 1-g substitution,
    bit-identical values), so both paths' relus merge into single wide
    Act instructions."""
    v = nc.vector
    xr = t["xh"][:, 2 + r0:2 + r1]
    rs = slice(r0, r1)
    v.tensor_tensor(t["cb"][:, rs], xr[:, :, 0:127], xr[:, :, 1:128],
                    Alu.is_ge)
    v.tensor_tensor(t["mb"][:, rs], xr[:, :, 0:127], xr[:, :, 1:128],
                    Alu.is_le)
    v.tensor_tensor(t["db"][:, rs], xr[:, :, 0:126], xr[:, :, 2:128],
                    Alu.is_ge)
    v.tensor_tensor(t["nb"][:, rs], xr[:, :, 0:126], xr[:, :, 2:128],
                    Alu.is_le)


def _t12_phase(nc, t, half, q0=0, q1=16):
    """Both paths' T = C*(D+P) + P*Q on rows [16*half+q0, 16*half+q1),
    double-width ops (path dim in the access pattern)."""
    v = nc.vector
    h0 = 16 * half
    r0, nr = h0 + q0, q1 - q0
    C, D = t["C12"], t["D12"]
    Pq = t["Pq"][half]
    Pp, Qp = Pq[:, :, 0, q0:q1], Pq[:, :, 1, q0:q1]
    Tc = t["Tb12"][:, :, r0:r0 + nr, 2:128]
    v.tensor_tensor(Tc, D[:, :, r0 + 2:r0 + nr + 2], Pp, Alu.add)
    v.tensor_tensor(Tc, Tc, C[:, :, r0 + 1:r0 + nr + 1], Alu.mult)
    v.tensor_tensor(Pp, Pp, Qp, Alu.mult)
    v.tensor_tensor(Tc, Tc, Pp, Alu.add)


def _z_phase(nc, t, psz1, psz2, uP, k, relu_dve=False):
    """One 4-row chunk's PE z-sums -> Act relus into SBUF U-tiles.
    relu_dve routes the relus to the (idle-by-then) DVE instead, used
    for the final chunks so the Act tail shrinks."""
    mm = nc.tensor.matmul
    a = nc.scalar
    rs = slice(k * CH, k * CH + CH)
    cb, db, mb, nb, Tb = t["cb"], t["db"], t["mb"], t["nb"], t["Tb12"]
    I, p3, n3 = t["I"], t["p3I"], t["n3I"]

    # each z half padded to 512 f32 = exactly one PSUM bank, so every
    # matmul accumulation group stays bank-aligned
    z1 = psz1.tile([128, 2, CH, 128], F32, tag="z1", name="z1")
    z2 = psz2.tile([128, 2, CH, 128], F32, tag="z2", name="z2")

    Tm = Tb[:, 0, rs, 2:128]
    Tn = Tb[:, 1, rs, 2:128]
    c0, c1 = cb[:, rs, 0:126], cb[:, rs, 1:127]
    m0, m1 = mb[:, rs, 0:126], mb[:, rs, 1:127]
    d0, n0 = db[:, rs, 0:126], nb[:, rs, 0:126]

    # z_U1 = T - 3c[v] + 3c[v+1]   (min path identical with c->m)
    mm(z1[:, 0, :, 0:126], I[:], Tm, start=True, stop=False)
    mm(z1[:, 0, :, 0:126], n3[:], c0, start=False, stop=False)
    mm(z1[:, 0, :, 0:126], p3[:], c1, start=False, stop=True)
    mm(z1[:, 1, :, 0:126], I[:], Tn, start=True, stop=False)
    mm(z1[:, 1, :, 0:126], n3[:], m0, start=False, stop=False)
    mm(z1[:, 1, :, 0:126], p3[:], m1, start=False, stop=True)
    # z_U2 = T - 3d[v] - 3c[v+1]   (min path identical with d->n, c->m)
    mm(z2[:, 0, :, 0:126], I[:], Tm, start=True, stop=False)
    mm(z2[:, 0, :, 0:126], n3[:], d0, start=False, stop=False)
    mm(z2[:, 0, :, 0:126], n3[:], c1, start=False, stop=True)
    mm(z2[:, 1, :, 0:126], I[:], Tn, start=True, stop=False)
    mm(z2[:, 1, :, 0:126], n3[:], n0, start=False, stop=False)
    mm(z2[:, 1, :, 0:126], n3[:], m1, start=False, stop=True)

    # both paths share each relu bias: two wide relus per chunk
    u1 = uP.tile([128, 2, CH, 126], BF16, tag="u1", name="u1")
    u2 = uP.tile([128, 2, CH, 126], BF16, tag="u2", name="u2")
    if relu_dve:
        nc.vector.tensor_scalar(u1[:], z1[:, :, :, 0:126], -3.0, 0.0,
                                Alu.add, Alu.max)
        nc.vector.tensor_scalar(u2[:], z2[:, :, :, 0:126], 0.0, None,
                                Alu.max)
    else:
        a.activation(u1[:], z1[:, :, :, 0:126], Act.Relu, bias=t["bn3"][:])
        a.activation(u2[:], z2[:, :, :, 0:126], Act.Relu, bias=0.0)
    return u1, u2


def _cnt_phase(nc, t, psc, k, Tb, u1, u2):
    """One chunk's 10-term PE count accumulation (emitted one chunk
    behind the z-phase so it never head-of-line blocks the in-order PE
    queue while waiting on the Act relus)."""
    mm = nc.tensor.matmul
    rs = slice(k * CH, k * CH + CH)
    I, nI = t["I"], t["nI"]
    cnt = psc.tile([128, CH, 128], F32, tag="cnt", name="cnt")
    # count[j] = T[j] - T'[j] - U1a[j] + U1a[j-1] - U2a[j] + U2a[j-2]
    #                         + U1b[j] - U1b[j-1] + U2b[j] - U2b[j-2]
    mm(cnt[:, :, 0:128], I[:], Tb[:, 0, rs, 2:130], start=True, stop=False)
    mm(cnt[:, :, 0:128], nI[:], Tb[:, 1, rs, 2:130], start=False, stop=False)
    mm(cnt[:, :, 0:126], nI[:], u1[:, 0], start=False, stop=False)
    mm(cnt[:, :, 1:127], I[:], u1[:, 0], start=False, stop=False)
    mm(cnt[:, :, 0:126], nI[:], u2[:, 0], start=False, stop=False)
    mm(cnt[:, :, 2:128], I[:], u2[:, 0], start=False, stop=False)
    mm(cnt[:, :, 0:126], I[:], u1[:, 1], start=False, stop=False)
    mm(cnt[:, :, 1:127], nI[:], u1[:, 1], start=False, stop=False)
    mm(cnt[:, :, 0:126], I[:], u2[:, 1], start=False, stop=False)
    mm(cnt[:, :, 2:128], nI[:], u2[:, 1], start=False, stop=True)
    return cnt


def _pipe_drain(nc, t, psc, op_, y_ap, zpend, cpend, zroom, croom):
    """Advance the trailing cnt / copy pipeline stages (interleaved so
    at most two cnt PSUM tiles are ever live)."""
    while len(zpend) > zroom:
        k0, rb, Tb0, a1, a2 = zpend.pop(0)
        cpend.append((k0, rb, _cnt_phase(nc, t, psc, k0, Tb0, a1, a2)))
    _copy_drain(nc, t, op_, y_ap, cpend, croom)


def _copy_drain(nc, t, op_, y_ap, cpend, croom):
    while len(cpend) > croom:
        k0, rb, cnt0 = cpend.pop(0)
        o8 = t.get("out8")
        if k0 % 2 == 0:
            o8 = op_.tile([128, 8, 128], mybir.dt.float32, tag="out8",
                          name="out8")
            t["out8"] = o8
        nc.scalar.copy(o8[:, (k0 % 2) * CH:(k0 % 2) * CH + CH], cnt0[:])
        if k0 % 2 == 1:
            nc.sync.dma_start(y_ap[:, rb + (k0 - 1) * CH:
                                   rb + (k0 - 1) * CH + 8], o8[:])


def _emit_kernel(tc, x_ap, y_ap):
    nc = tc.nc
    with (
        tc.tile_pool(name="io", bufs=2) as io,
        tc.tile_pool(name="tb", bufs=2) as tbp,
        tc.tile_pool(name="out", bufs=2) as op_,
        tc.tile_pool(name="msk", bufs=1) as mk,
        tc.tile_pool(name="u", bufs=7) as uP,
        tc.psum_pool(name="ps", bufs=2) as ps1,
        tc.psum_pool(name="psb", bufs=1) as ps1b,
        tc.psum_pool(name="pc", bufs=2) as ps2,
    ):
        t = {
            "S": mk.tile([128, 36, 126], F16, tag="S", name="S"),
            "C12": mk.tile([128, 2, 33, 126], BF16, tag="C12", name="C12"),
            "D12": mk.tile([128, 2, 34, 126], BF16, tag="D12", name="D12"),
            # Pq[half][:, path, 0]=P, [:, path, 1]=Q  (16-row halves)
            "Pq": [mk.tile([128, 2, 2, 16, 126], BF16, tag=f"Pq{h}",
                           name=f"Pq{h}") for h in range(2)],
            "cb": mk.tile([128, RB, 127], BF16, tag="cb", name="cb"),
            "db": mk.tile([128, RB, 126], BF16, tag="db", name="db"),
            "mb": mk.tile([128, RB, 127], BF16, tag="mb", name="mb"),
            "nb": mk.tile([128, RB, 126], BF16, tag="nb", name="nb"),
        }
        # kick off block 0's input chunks before the const setup so the
        # SWDGE cast DMAs overlap the GpSimd memsets
        xh0 = io.tile([128, 36, 128], F16, tag="xh", name="xh")
        for a0, a1 in ((0, 6), (6, 16), (16, 26), (26, 34)):
            nc.gpsimd.dma_start(xh0[:, 2 + a0:2 + a1], x_ap[:, a0:a1])

        for nm, fill in (("I", 1.0), ("nI", -1.0), ("p3I", 3.0),
                         ("n3I", -3.0)):
            ap_ = mk.tile([128, 128], BF16, tag=nm, name=nm)
            nc.gpsimd.memset(ap_, 0.0)
            nc.gpsimd.affine_select(
                out=ap_, in_=ap_, compare_op=Alu.not_equal, fill=fill,
                base=0, pattern=[[-1, 128]], channel_multiplier=1,
            )
            t[nm] = ap_
        for nm, fill in (("bn3", -3.0), ("bn6", -6.0)):
            ap_ = mk.tile([128, 1], F32, tag=nm, name=nm)
            nc.gpsimd.memset(ap_, fill)
            t[nm] = ap_

        zpend = []   # (k, r0, Tb, u1, u2) awaiting cnt emission
        cpend = []   # (k, r0, cnt) awaiting Act copy + DMA
        for blk in range(NBLK):
            r0 = blk * RB
            lo, hi = max(r0 - 2, 0), min(r0 + RB + 2, H)
            s0 = lo - (r0 - 2)
            nrows = hi - lo
            # input loaded with in-flight f32->fp16 cast (GpSimd SWDGE):
            # no staging buffer, no Act convert (block 0 was issued
            # before the const setup)
            if blk == 0:
                xh = xh0
            else:
                xh = io.tile([128, 36, 128], F16, tag="xh", name="xh")
                for a0, a1 in ((0, 18), (18, nrows)):
                    nc.gpsimd.dma_start(xh[:, s0 + a0:s0 + a1],
                                        x_ap[:, lo + a0:lo + a1])
            t["xh"] = xh

            Tb = tbp.tile([128, 2, RB, 130], BF16, tag="Tb12", name="Tb12")
            t["Tb12"] = Tb
            if blk < 2:
                nc.gpsimd.memset(Tb[:, :, :, 0:2], 0.0)
                nc.gpsimd.memset(Tb[:, :, :, 128:130], 0.0)

            # emit pending cnt/copy stages from the previous block FIRST
            # so they sit ahead of this block's work in the in-order PE
            # and Act queues (they are data-ready; this block's z-groups
            # are not until t12)
            _pipe_drain(nc, t, ps2, op_, y_ap, zpend, cpend, 0, 0)

            _cmp_phase(nc, t, blk, True)
            _cmp_phase(nc, t, blk, False)
            _h_phase(nc, t, 0, 8)

            # combine as a 3-stage software pipeline (persisting across
            # blocks): the PE cnt group for chunk k and the Act copy
            # both trail one chunk behind the z/relu stage, so neither
            # the in-order PE queue nor the in-order Act sequencer ever
            # head-of-line blocks on a not-yet-ready dependency.
            # Block 0's first half and the last block run t12 at 8-row
            # granularity (earlier PE start / shorter tail).
            first = True
            for half in range(2):
                fine = (blk == 0 and half == 0) or blk == NBLK - 1
                quarters = ((0, 8), (8, 16)) if fine else ((0, 16),)
                for q0, q1 in quarters:
                    _t12_phase(nc, t, half, q0, q1)
                    if first:
                        # rest of the horizontal masks after the first
                        # t12 quarter so the combine starts sooner
                        _h_phase(nc, t, 8, RB)
                        first = False
                    k0 = (half * 16 + q0) // CH
                    for k in range(k0, k0 + (q1 - q0) // CH):
                        u1, u2 = _z_phase(nc, t, ps1, ps1b, uP, k)
                        zpend.append((k, r0, Tb, u1, u2))
                        _pipe_drain(nc, t, ps2, op_, y_ap, zpend, cpend,
                                    6, 1)
        _pipe_drain(nc, t, ps2, op_, y_ap, zpend, cpend, 0, 0)


_NC_CACHE = {}


def _build():
    if "nc" in _NC_CACHE:
        return _NC_CACHE["nc"]
    nc = bacc.Bacc(
        "TRN2",
        target_bir_lowering=False,
        debug=False,
        enable_asserts=False,
        num_devices=NCORES,
    )
    x_d = nc.dram_tensor("x", [P, H, W], F32, kind="ExternalInput")
    y_d = nc.dram_tensor("y", [P, H, W], F32, kind="ExternalOutput")
    with tile.TileContext(nc) as tc:
        _emit_kernel(tc, x_d.ap(), y_d.ap())
    nc.compile()
    _NC_CACHE["nc"] = nc
    return nc


def run(x, **spmd_kwargs):
    nc = _build()
    xf = np.ascontiguousarray(np.asarray(x, dtype=np.float32).reshape(N_ * C_, H, W))
    in_maps = [{"x": xf[k * P:(k + 1) * P]} for k in range(NCORES)]
    # the runtime occasionally hits a transient NRT exec error; one
    # retry recovers it
    try:
        res = bass_utils.run_bass_kernel_spmd(
            nc, in_maps, core_ids=list(range(NCORES)), **spmd_kwargs
        )
    except Exception:
        import time as _time

        _time.sleep(5)
        res = bass_utils.run_bass_kernel_spmd(
            nc, in_maps, core_ids=list(range(NCORES)), **spmd_kwargs
        )
    out = np.concatenate([res.results[k]["y"] for k in range(NCORES)], axis=0)
    return out.reshape(N_, C_, H, W), res


def kernel(x):
    out, _ = run(x)
    return out
